# revision 31
# baseline (speedup 1.0000x reference)
# Trainium2 Bass kernel for nn_BinLinearEval:
#   out[b, o] = (round(x @ W.T + bias) * sign >= 0) ? 1.0 : 0.0
#
# Math folding (exact because bias is integer-valued and sign in {-1,+1}):
#   out = 1  iff  sign*(dot + bias) >= -0.5
#       = 1  iff  dot' >= thr_o      where dot' = x @ (sign.T*W).T  (W' still
#         ternary, exact in fp16) and thr_o = -sign_o*bias_o - 0.5.
# The device computes dot' in two accumulated passes — an fp16 hi pass plus
# an fp8-e4m3 DoubleRow residual pass (x_lo*2^6 vs W'*2^-6, both exactly
# representable; DoubleRow contracts K=256 per matmul at ~1.75x the fp16
# rate) — giving near-fp32 accuracy (20/16.7M threshold flips) at ~60% of
# the 2xfp16 cost. Epilogue is a single per-partition is_ge threshold.
#
# Sharding: data-parallel over batch, 8192 rows per core. x is pre-transposed
# on the host to [feature, batch] layout so the contract dim lands on SBUF
# partitions; output is produced as [out, batch] per core and re-assembled /
# transposed on the host.

import os
from contextlib import ExitStack

import numpy as np
import ml_dtypes

BATCH, IN_F, OUT_F = 65536, 1024, 256
N_CORES = 8
B_CORE = BATCH // N_CORES  # 8192
P = 128
KC = IN_F // P             # 8 k-chunks
OC = OUT_F // P            # 2 out-channel chunks
BT = 512                   # matmul moving free dim
# Uniform small groups + deep buffering: DMA stays saturated and the PE
# never outruns the prefetch pipeline by more than the buffer depth.
GROUPS = [512] * (B_CORE // 512)
assert sum(GROUPS) == B_CORE
IO_BUFS = 6

_CACHE = {}


def _build():
    """Build (and cache) the Bass module. Returns the compiled nc."""
    if "nc" in _CACHE:
        return _CACHE["nc"]

    import concourse.bacc as bacc
    import concourse.mybir as mybir
    import concourse.tile as tile

    nc = bacc.Bacc(
        "TRN2",
        target_bir_lowering=False,
        debug=False,
        num_devices=N_CORES,
    )

    f16 = mybir.dt.float16
    f32 = mybir.dt.float32
    bf16 = mybir.dt.bfloat16
    f8 = mybir.dt.float8e4

    # group-major layouts: one group's slab is contiguous per partition
    # (8 KB / 4 KB descriptors instead of 1 KB / 512 B strided rows)
    n_groups = len(GROUPS)
    xhi_d = nc.dram_tensor(
        "xhi", [P, n_groups, KC, GROUPS[0]], f16, kind="ExternalInput"
    ).ap()
    xlo_d = nc.dram_tensor(
        "xlo8", [P, n_groups, KC // 2, 2, GROUPS[0]], f8, kind="ExternalInput"
    ).ap()
    wt_d = nc.dram_tensor("wt", [P, KC, OUT_F], f16, kind="ExternalInput").ap()
    wlo_d = nc.dram_tensor(
        "wlo8", [P, KC // 2, 2, OUT_F], f8, kind="ExternalInput"
    ).ap()
    thr_d = nc.dram_tensor("thr", [P, OC], f32, kind="ExternalInput").ap()
    out_d = nc.dram_tensor("out", [OC, P, B_CORE], bf16, kind="ExternalOutput").ap()

    with tile.TileContext(nc) as tc, ExitStack() as ctx:
        const = ctx.enter_context(tc.tile_pool(name="const", bufs=1))
        io = ctx.enter_context(tc.tile_pool(name="io", bufs=IO_BUFS))
        outp = ctx.enter_context(tc.tile_pool(name="outp", bufs=4))
        psum = ctx.enter_context(tc.tile_pool(name="psum", bufs=4, space="PSUM"))

        # consts ride the ACT HWDGE ring so the SP ring can start streaming
        # the first x group immediately; first matmul waits on whichever
        # finishes later (~2.8us instead of ~4.9us serialized)
        wt_sb = const.tile([P, KC, OUT_F], f16)
        nc.scalar.dma_start(out=wt_sb, in_=wt_d)
        wlo_sb = const.tile([P, KC // 2, 2, OUT_F], f8)
        nc.scalar.dma_start(out=wlo_sb, in_=wlo_d)
        thr_sb = const.tile([P, OC], f32)
        nc.scalar.dma_start(out=thr_sb, in_=thr_d)

        g0 = 0
        for g, group in enumerate(GROUPS):
            if g == 0:
                # split group 0's hi DMA by k-quarters: the first matmuls
                # gate on 0.25 MB (+ completion receipt) instead of 1 MB,
                # starting the PE a few us earlier
                QK = KC // 4
                xh0 = []
                for q in range(4):
                    t = io.tile(
                        [P, QK, max(GROUPS)], f16, name=f"xh0_{q}", bufs=1
                    )
                    nc.sync.dma_start(
                        out=t, in_=xhi_d[:, 0, q * QK : (q + 1) * QK]
                    )
                    xh0.append(t)

                def hi_ap(k, lo_, hi_):
                    return xh0[k // QK][:, k % QK, lo_:hi_]
            else:
                xhi_sb = io.tile([P, KC, max(GROUPS)], f16, name="xhi_sb")[
                    :, :, :group
                ]
                nc.sync.dma_start(out=xhi_sb, in_=xhi_d[:, g])

                def hi_ap(k, lo_, hi_, t=xhi_sb):
                    return t[:, k, lo_:hi_]

            xlo_sb = io.tile([P, KC // 2, 2, max(GROUPS)], f8, name="xlo_sb")[
                :, :, :, :group
            ]
            if g == 0:
                # group 0's lo rides the ACT ring (behind the small consts)
                # instead of queuing behind group 0's hi on the SP ring
                nc.scalar.dma_start(out=xlo_sb, in_=xlo_d[:, g])
            else:
                nc.sync.dma_start(out=xlo_sb, in_=xlo_d[:, g])
            for bt in range(group // BT):
                b0 = bt * BT
                for oc in range(OC):
                    ps = psum.tile([P, BT], f32, name="ps")
                    # all-hi then all-lo: the first matmuls of the kernel
                    # only need the hi half of the first group in SBUF
                    for k in range(KC):
                        nc.tensor.matmul(
                            ps,
                            wt_sb[:, k, oc * P : (oc + 1) * P],
                            hi_ap(k, b0, b0 + BT),
                            start=(k == 0),
                            stop=False,
                        )
                    # lo pass: fp8 e4m3 DoubleRow, contracts 256 per matmul
                    for c in range(KC // 2):
                        nc.tensor.matmul(
                            ps,
                            wlo_sb[:, c, :, oc * P : (oc + 1) * P],
                            xlo_sb[:, c, :, b0 : b0 + BT],
                            start=False,
                            stop=(c == KC // 2 - 1),
                            perf_mode=mybir.MatmulPerfMode.DoubleRow,
                        )
                    ob = outp.tile([P, BT], bf16, name="ob")
                    nc.vector.tensor_scalar(
                        ob,
                        ps,
                        thr_sb[:, oc : oc + 1],
                        None,
                        mybir.AluOpType.is_ge,
                    )
                    # out-DMAs ride the ACT HWDGE ring so they never block
                    # the input-DMA FIFO on the SP ring
                    nc.scalar.dma_start(
                        out=out_d[oc, :, g0 + b0 : g0 + b0 + BT], in_=ob
                    )
            g0 += group

    nc.compile()
    _CACHE["nc"] = nc
    return nc


def _prep_inputs(x, weight, bias, sign):
    """Host-side prep: fold sign into weights, build thresholds, split x into
    fp16 hi/lo, transpose to [feature, batch] per-core tiles."""
    x = np.asarray(x, dtype=np.float32)
    weight = np.asarray(weight, dtype=np.float32)
    bias = np.asarray(bias, dtype=np.float32)
    sign = np.asarray(sign, dtype=np.float32).reshape(1, OUT_F)

    wp = sign.T * weight                      # [OUT_F, IN_F], ternary
    thr = (-sign[0] * bias - np.float32(0.5)).astype(np.float32)  # [OUT_F]

    wt = np.ascontiguousarray(
        wp.T.reshape(KC, P, OUT_F).transpose(1, 0, 2)
    ).astype(np.float16)                      # [P, KC, OUT_F]
    thr2 = np.ascontiguousarray(thr.reshape(OC, P).T)  # [P, OC]

    xhi = x.astype(np.float16)
    f8np = ml_dtypes.float8_e4m3fn
    xlo8 = ((x - xhi.astype(np.float32)) * np.float32(64.0)).astype(f8np)
    wlo8 = np.ascontiguousarray(
        (wp.T * np.float32(1.0 / 64.0))
        .reshape(KC // 2, 2, P, OUT_F)
        .transpose(2, 0, 1, 3)
    ).astype(f8np)                            # [P, KC//2, 2, OUT_F]

    n_groups = len(GROUPS)
    grp = GROUPS[0]
    in_maps = []
    for c in range(N_CORES):
        sl = slice(c * B_CORE, (c + 1) * B_CORE)
        hi = np.ascontiguousarray(
            xhi[sl].reshape(n_groups, grp, KC, P).transpose(3, 0, 2, 1)
        )                                      # [P, n_groups, KC, grp]
        lo = np.ascontiguousarray(
            xlo8[sl]
            .reshape(n_groups, grp, KC // 2, 2, P)
            .transpose(4, 0, 2, 3, 1)
        )                                      # [P, n_groups, KC//2, 2, grp]
        in_maps.append(
            {"xhi": hi, "xlo8": lo, "wt": wt, "wlo8": wlo8, "thr": thr2}
        )
    return in_maps


def _assemble(results):
    """[core][OC, P, B_CORE] bf16 -> [BATCH, OUT_F] fp32"""
    full = np.concatenate(
        [r["out"].reshape(OUT_F, B_CORE) for r in results], axis=1
    )  # [OUT_F, BATCH]
    return np.ascontiguousarray(full.T).astype(np.float32)


def run(x, weight, bias, sign, trace=False):
    """Run the kernel; returns (output, BassKernelResults)."""
    from concourse.bass_utils import run_bass_kernel_spmd

    if not trace:
        # The NTFF profile hook module may be absent in this image; make
        # sure a stray BASS_TRACE=1 can't route us into the trace path.
        os.environ["BASS_NEVER_TRACE"] = "1"
    else:
        os.environ.pop("BASS_NEVER_TRACE", None)

    nc = _build()
    in_maps = _prep_inputs(x, weight, bias, sign)
    res = run_bass_kernel_spmd(
        nc,
        in_maps,
        core_ids=list(range(N_CORES)),
        trace=trace,
    )
    return _assemble(res.results), res


def kernel(x, weight, bias, sign):
    out, _ = run(x, weight, bias, sign, trace=False)
    return out


# revision 35
# speedup vs baseline: 1.0080x; 1.0080x over previous
# Trainium2 Bass kernel for nn_BinLinearEval:
#   out[b, o] = (round(x @ W.T + bias) * sign >= 0) ? 1.0 : 0.0
#
# Math folding (exact because bias is integer-valued and sign in {-1,+1}):
#   out = 1  iff  sign*(dot + bias) >= -0.5
#       = 1  iff  dot' >= thr_o      where dot' = x @ (sign.T*W).T  (W' still
#         ternary, exact in fp16) and thr_o = -sign_o*bias_o - 0.5.
# The device computes dot' in two accumulated passes — an fp16 hi pass plus
# an fp8-e4m3 DoubleRow residual pass (x_lo*2^6 vs W'*2^-6, both exactly
# representable; DoubleRow contracts K=256 per matmul at ~1.75x the fp16
# rate) — giving near-fp32 accuracy (20/16.7M threshold flips) at ~60% of
# the 2xfp16 cost. Epilogue is a single per-partition is_ge threshold.
#
# Sharding: data-parallel over batch, 8192 rows per core. x is pre-transposed
# on the host to [feature, batch] layout so the contract dim lands on SBUF
# partitions; output is produced as [out, batch] per core and re-assembled /
# transposed on the host.

import os
from contextlib import ExitStack

import numpy as np
import ml_dtypes

BATCH, IN_F, OUT_F = 65536, 1024, 256
N_CORES = 8
B_CORE = BATCH // N_CORES  # 8192
P = 128
KC = IN_F // P             # 8 k-chunks
OC = OUT_F // P            # 2 out-channel chunks
BT = 512                   # matmul moving free dim
# Uniform small groups + deep buffering: DMA stays saturated and the PE
# never outruns the prefetch pipeline by more than the buffer depth.
GROUPS = [512] * (B_CORE // 512)
assert sum(GROUPS) == B_CORE
IO_BUFS = 6

_CACHE = {}


def _build():
    """Build (and cache) the Bass module. Returns the compiled nc."""
    if "nc" in _CACHE:
        return _CACHE["nc"]

    import concourse.bacc as bacc
    import concourse.mybir as mybir
    import concourse.tile as tile

    nc = bacc.Bacc(
        "TRN2",
        target_bir_lowering=False,
        debug=False,
        num_devices=N_CORES,
    )

    f16 = mybir.dt.float16
    f32 = mybir.dt.float32
    bf16 = mybir.dt.bfloat16
    f8 = mybir.dt.float8e4

    # group-major layouts: one group's slab is contiguous per partition
    # (8 KB / 4 KB descriptors instead of 1 KB / 512 B strided rows)
    n_groups = len(GROUPS)
    xhi_d = nc.dram_tensor(
        "xhi", [P, n_groups, KC, GROUPS[0]], f16, kind="ExternalInput"
    ).ap()
    xlo_d = nc.dram_tensor(
        "xlo8", [P, n_groups, KC // 2, 2, GROUPS[0]], f8, kind="ExternalInput"
    ).ap()
    wt_d = nc.dram_tensor("wt", [P, KC, OUT_F], f16, kind="ExternalInput").ap()
    wlo_d = nc.dram_tensor(
        "wlo8", [P, KC // 2, 2, OUT_F], f8, kind="ExternalInput"
    ).ap()
    thr_d = nc.dram_tensor("thr", [P, OC], f32, kind="ExternalInput").ap()
    out_d = nc.dram_tensor("out", [OC, P, B_CORE], bf16, kind="ExternalOutput").ap()

    with tile.TileContext(nc) as tc, ExitStack() as ctx:
        const = ctx.enter_context(tc.tile_pool(name="const", bufs=1))
        io = ctx.enter_context(tc.tile_pool(name="io", bufs=IO_BUFS))
        outp = ctx.enter_context(tc.tile_pool(name="outp", bufs=4))
        psum = ctx.enter_context(tc.tile_pool(name="psum", bufs=4, space="PSUM"))

        # consts ride the ACT HWDGE ring so the SP ring can start streaming
        # the first x group immediately; first matmul waits on whichever
        # finishes later (~2.8us instead of ~4.9us serialized)
        wt_sb = const.tile([P, KC, OUT_F], f16)
        nc.scalar.dma_start(out=wt_sb, in_=wt_d)
        wlo_sb = const.tile([P, KC // 2, 2, OUT_F], f8)
        nc.scalar.dma_start(out=wlo_sb, in_=wlo_d)
        thr_sb = const.tile([P, OC], f32)
        nc.scalar.dma_start(out=thr_sb, in_=thr_d)

        g0 = 0
        for g, group in enumerate(GROUPS):
            if g == 0:
                # split group 0's hi DMA by k-halves: the first matmuls gate
                # on 0.5 MB (+ completion receipt) instead of 1 MB, starting
                # the PE a few us earlier (quarters tested worse: per-DMA
                # completion receipts serialize)
                xh0a = io.tile(
                    [P, KC // 2, max(GROUPS)], f16, name="xh0a", bufs=1
                )
                xh0b = io.tile(
                    [P, KC // 2, max(GROUPS)], f16, name="xh0b", bufs=1
                )
                nc.sync.dma_start(out=xh0a, in_=xhi_d[:, 0, : KC // 2])
                nc.sync.dma_start(out=xh0b, in_=xhi_d[:, 0, KC // 2 :])

                def hi_ap(k, lo_, hi_):
                    t = xh0a if k < KC // 2 else xh0b
                    return t[:, k % (KC // 2), lo_:hi_]
            else:
                xhi_sb = io.tile([P, KC, max(GROUPS)], f16, name="xhi_sb")[
                    :, :, :group
                ]
                nc.sync.dma_start(out=xhi_sb, in_=xhi_d[:, g])

                def hi_ap(k, lo_, hi_, t=xhi_sb):
                    return t[:, k, lo_:hi_]

            if g == 0:
                # same trick for group 0's lo: halves, so the first
                # DoubleRow matmul gates on 0.25 MB + one receipt
                QC = KC // 4
                xl0a = io.tile(
                    [P, QC, 2, max(GROUPS)], f8, name="xl0a", bufs=1
                )
                xl0b = io.tile(
                    [P, QC, 2, max(GROUPS)], f8, name="xl0b", bufs=1
                )
                nc.sync.dma_start(out=xl0a, in_=xlo_d[:, 0, :QC])
                nc.sync.dma_start(out=xl0b, in_=xlo_d[:, 0, QC:])

                def lo_ap(c, lo_, hi_):
                    t = xl0a if c < QC else xl0b
                    return t[:, c % QC, :, lo_:hi_]
            else:
                xlo_sb = io.tile(
                    [P, KC // 2, 2, max(GROUPS)], f8, name="xlo_sb"
                )[:, :, :, :group]
                nc.sync.dma_start(out=xlo_sb, in_=xlo_d[:, g])

                def lo_ap(c, lo_, hi_, t=xlo_sb):
                    return t[:, c, :, lo_:hi_]
            for bt in range(group // BT):
                b0 = bt * BT
                for oc in range(OC):
                    ps = psum.tile([P, BT], f32, name="ps")
                    # all-hi then all-lo: the first matmuls of the kernel
                    # only need the hi half of the first group in SBUF
                    for k in range(KC):
                        nc.tensor.matmul(
                            ps,
                            wt_sb[:, k, oc * P : (oc + 1) * P],
                            hi_ap(k, b0, b0 + BT),
                            start=(k == 0),
                            stop=False,
                        )
                    # lo pass: fp8 e4m3 DoubleRow, contracts 256 per matmul
                    for c in range(KC // 2):
                        nc.tensor.matmul(
                            ps,
                            wlo_sb[:, c, :, oc * P : (oc + 1) * P],
                            lo_ap(c, b0, b0 + BT),
                            start=False,
                            stop=(c == KC // 2 - 1),
                            perf_mode=mybir.MatmulPerfMode.DoubleRow,
                        )
                    ob = outp.tile([P, BT], bf16, name="ob")
                    nc.vector.tensor_scalar(
                        ob,
                        ps,
                        thr_sb[:, oc : oc + 1],
                        None,
                        mybir.AluOpType.is_ge,
                    )
                    # out-DMAs ride the ACT HWDGE ring so they never block
                    # the input-DMA FIFO on the SP ring
                    nc.scalar.dma_start(
                        out=out_d[oc, :, g0 + b0 : g0 + b0 + BT], in_=ob
                    )
            g0 += group

    nc.compile()
    _CACHE["nc"] = nc
    return nc


def _prep_inputs(x, weight, bias, sign):
    """Host-side prep: fold sign into weights, build thresholds, split x into
    fp16 hi/lo, transpose to [feature, batch] per-core tiles."""
    x = np.asarray(x, dtype=np.float32)
    weight = np.asarray(weight, dtype=np.float32)
    bias = np.asarray(bias, dtype=np.float32)
    sign = np.asarray(sign, dtype=np.float32).reshape(1, OUT_F)

    wp = sign.T * weight                      # [OUT_F, IN_F], ternary
    thr = (-sign[0] * bias - np.float32(0.5)).astype(np.float32)  # [OUT_F]

    wt = np.ascontiguousarray(
        wp.T.reshape(KC, P, OUT_F).transpose(1, 0, 2)
    ).astype(np.float16)                      # [P, KC, OUT_F]
    thr2 = np.ascontiguousarray(thr.reshape(OC, P).T)  # [P, OC]

    xhi = x.astype(np.float16)
    f8np = ml_dtypes.float8_e4m3fn
    xlo8 = ((x - xhi.astype(np.float32)) * np.float32(64.0)).astype(f8np)
    wlo8 = np.ascontiguousarray(
        (wp.T * np.float32(1.0 / 64.0))
        .reshape(KC // 2, 2, P, OUT_F)
        .transpose(2, 0, 1, 3)
    ).astype(f8np)                            # [P, KC//2, 2, OUT_F]

    n_groups = len(GROUPS)
    grp = GROUPS[0]
    in_maps = []
    for c in range(N_CORES):
        sl = slice(c * B_CORE, (c + 1) * B_CORE)
        hi = np.ascontiguousarray(
            xhi[sl].reshape(n_groups, grp, KC, P).transpose(3, 0, 2, 1)
        )                                      # [P, n_groups, KC, grp]
        lo = np.ascontiguousarray(
            xlo8[sl]
            .reshape(n_groups, grp, KC // 2, 2, P)
            .transpose(4, 0, 2, 3, 1)
        )                                      # [P, n_groups, KC//2, 2, grp]
        in_maps.append(
            {"xhi": hi, "xlo8": lo, "wt": wt, "wlo8": wlo8, "thr": thr2}
        )
    return in_maps


def _assemble(results):
    """[core][OC, P, B_CORE] bf16 -> [BATCH, OUT_F] fp32"""
    full = np.concatenate(
        [r["out"].reshape(OUT_F, B_CORE) for r in results], axis=1
    )  # [OUT_F, BATCH]
    return np.ascontiguousarray(full.T).astype(np.float32)


def run(x, weight, bias, sign, trace=False):
    """Run the kernel; returns (output, BassKernelResults)."""
    from concourse.bass_utils import run_bass_kernel_spmd

    if not trace:
        # The NTFF profile hook module may be absent in this image; make
        # sure a stray BASS_TRACE=1 can't route us into the trace path.
        os.environ["BASS_NEVER_TRACE"] = "1"
    else:
        os.environ.pop("BASS_NEVER_TRACE", None)

    nc = _build()
    in_maps = _prep_inputs(x, weight, bias, sign)
    res = run_bass_kernel_spmd(
        nc,
        in_maps,
        core_ids=list(range(N_CORES)),
        trace=trace,
    )
    return _assemble(res.results), res


def kernel(x, weight, bias, sign):
    out, _ = run(x, weight, bias, sign, trace=False)
    return out
